# revision 13
# baseline (speedup 1.0000x reference)
"""ChaoticEvolutionGNN on 8 TRN2 NeuronCores (bass/Tile).

Nodes are degree-sorted and snake-dealt to 8 cores (1250 each, padded to
1280 = 10 tiles of 128).  Dense phase (encoder / fractal stack / evo gate)
is node-sharded with replicated weights; the jax threefry noise terms are
precomputed on host as additive Z = cf*noise@W tensors.  Per layer the
packed rows [hm | a_i | a_j] are AllGathered into a DRAM table; the GAT
edge phase is dst-sharded: each core dma_gathers its in-edges' source rows
(graph structure is compile-time constant -> host-precomputed slot indices
and masks), computes the segment softmax unnormalized, and folds the
normalization into the head-mean.
"""

import os
import numpy as np
import ml_dtypes

_NO_GATHER = os.environ.get("KNOGATHER", "0") == "1"
_NO_COLL = os.environ.get("KNOCOLL", "0") == "1"
_NO_TRANSP = os.environ.get("KNOTRANSP", "0") == "1"  # unused placeholder

N, E, F_IN, H, HEADS, L, DEPTH = 10000, 160000, 128, 64, 8, 6, 3
M = HEADS * H
OUT_DIM = 64
EPS = 1e-5
CHAOS = 0.1

NC = 8
NL = 1280
NREAL = 1250
T = NL // 128
ROW = 640          # packed row in bf16: hm 512 | a_i 8 | a_j 8 | pad
KCH = 12           # gather chunk depth

_CACHE = {}


# ------------------------------------------------------------------ host prep

def _noise_terms(tW1, tW2, tb1, tb2):
    import jax
    cpu = jax.devices("cpu")[0]
    nkey = jax.random.key(42)
    Z = np.zeros((12, N, M), np.float32)
    with jax.default_device(cpu):
        for i in range(L):
            cf = CHAOS * (1.0 + 0.1 * i)
            for d in (1, 2):
                nz = np.asarray(
                    jax.random.normal(jax.random.fold_in(nkey, i * 10 + (d - 1)),
                                      (N, M), np.float32), np.float32)
                W = np.asarray(tW1[i] if d == 1 else tW2[i], np.float32)
                b = np.asarray(tb1[i] if d == 1 else tb2[i], np.float32)
                Z[i * 2 + (d - 1)] = cf * (nz @ W) + b
    return Z


def _partition(edge_index):
    src = np.asarray(edge_index[0], np.int64)
    dst = np.asarray(edge_index[1], np.int64)
    deg = np.bincount(dst, minlength=N)
    order = np.argsort(-deg, kind="stable")
    core_of = np.empty(N, np.int32)
    local_of = np.empty(N, np.int32)
    cnt = np.zeros(NC, np.int32)
    for r in range(N):
        n = order[r]
        b = (r // NC) % 2
        c = (r % NC) if b == 0 else (NC - 1 - (r % NC))
        core_of[n] = c
        local_of[n] = cnt[c]
        cnt[c] += 1
    assert (cnt == NREAL).all()

    adj = [[[] for _ in range(NL)] for _ in range(NC)]
    for s, d in zip(src, dst):
        adj[core_of[d]][local_of[d]].append(int(s))

    D = []
    for t in range(T):
        m = 1
        for c in range(NC):
            base = t * 128
            for p in range(128):
                m = max(m, len(adj[c][base + p]))
        D.append(m)

    chunks = []
    for t in range(T):
        k = 0
        while k < D[t]:
            kc = min(KCH, D[t] - k)
            chunks.append((t, k, kc))
            k += kc

    S = sum(D)
    tbase = np.concatenate([[0], np.cumsum(D)]).astype(int)
    gidx = np.zeros((NC, 128, 8 * S), np.int16)
    emask = np.zeros((NC, 128, 1, S), ml_dtypes.bfloat16)
    for c in range(NC):
        col = 0
        for (t, k0, kc) in chunks:
            n = 128 * kc
            flat = np.zeros(n, np.int64)
            for kk in range(kc):
                k = k0 + kk
                for p in range(128):
                    lst = adj[c][t * 128 + p]
                    if k < len(lst):
                        s = lst[k]
                        ls = local_of[s]
                        flat[kk * 128 + p] = core_of[s] * NL + (ls % 128) * T + ls // 128
                        emask[c, p, 0, tbase[t] + k] = 1.0
            w = np.zeros((16, n // 16), np.int64)
            for j in range(n):
                w[j % 16, j // 16] = flat[j]
            gidx[c, :, col:col + n // 16] = np.tile(w.astype(np.int16), (8, 1))
            col += n // 16
    meta = dict(D=D, chunks=chunks, S=S, tbase=tbase)
    return meta, gidx, emask, core_of, local_of


def _bf(x):
    return np.ascontiguousarray(np.asarray(x, np.float32)).astype(ml_dtypes.bfloat16)


def _prep(inputs):
    ei = np.asarray(inputs["edge_index"])
    meta, gidx, emask, core_of, local_of = _partition(ei)

    perm = np.zeros((NC, NREAL), np.int64)
    for n in range(N):
        perm[core_of[n], local_of[n]] = n

    x = np.asarray(inputs["x"], np.float32)
    Z = _noise_terms(inputs["tW1"], inputs["tW2"], inputs["tb1"], inputs["tb2"])

    tW0 = np.asarray(inputs["tW0"], np.float32)
    tW1 = np.asarray(inputs["tW1"], np.float32)
    tW2 = np.asarray(inputs["tW2"], np.float32)
    att = np.asarray(inputs["att"], np.float32)

    W0 = np.transpose(tW0, (1, 0, 2)).copy()              # [64, L, 512]
    Wk = np.zeros((128, L, 2, 4, M), np.float32)
    for i in range(L):
        for dd, tw in enumerate((tW1, tW2)):
            for ko in range(4):
                Wk[:, i, dd, ko, :] = tw[i][ko * 128:(ko + 1) * 128, :]
    Watt = np.zeros((128, L, 4, 16), np.float32)
    for i in range(L):
        for h in range(HEADS):
            for cc in range(H):
                k = h * H + cc
                Watt[k % 128, i, k // 128, h] = att[i, h, cc]
                Watt[k % 128, i, k // 128, 8 + h] = att[i, h, H + cc]
    Wdec = np.concatenate([
        np.asarray(inputs["dec_state_W"], np.float32),
        np.asarray(inputs["dec_imp_W"], np.float32),
        np.asarray(inputs["dec_chaos_W"], np.float32),
        np.asarray(inputs["dec_evo_W"], np.float32)], axis=1)
    bdec = np.concatenate([
        np.ravel(inputs["dec_state_b"]), np.ravel(inputs["dec_imp_b"]),
        np.ravel(inputs["dec_chaos_b"]), np.ravel(inputs["dec_evo_b"])])

    nz = lambda a: bool(np.any(np.asarray(a)))
    if (nz(inputs["enc_b"]) or nz(inputs["evo_b1"]) or nz(inputs["evo_b2"])
            or nz(bdec) or nz(inputs["tb0"]) or nz(inputs["enc_beta"])
            or nz(inputs["tbeta0"]) or nz(inputs["tbeta1"]) or nz(inputs["tbeta2"])
            or not np.allclose(np.asarray(inputs["enc_g"]), 1.0)
            or not np.allclose(np.asarray(inputs["tg0"]), 1.0)
            or not np.allclose(np.asarray(inputs["tg1"]), 1.0)
            or not np.allclose(np.asarray(inputs["tg2"]), 1.0)):
        raise NotImplementedError("nonzero affine/bias path not built")

    chaos128 = np.tile((np.asarray(inputs["chaos_memory"], np.float32)
                        * CHAOS).reshape(1, H), (128, 1))
    shared = dict(W0=_bf(W0), Wk=_bf(Wk), Watt=_bf(Watt),
                  Wenc=_bf(inputs["enc_W"]), Wevo1=_bf(inputs["evo_W1"]),
                  Wevo2=_bf(inputs["evo_W2"]), Wdec=_bf(Wdec),
                  chaos128=chaos128)
    in_maps = []
    for c in range(NC):
        g = perm[c]
        xT = np.zeros((128, NL), np.float32)
        xT[:, :NREAL] = x[g].T
        Zc = np.zeros((12, NL, M), np.float32)
        Zc[:, :NREAL, :] = Z[:, g, :]
        in_maps.append(dict(xT=xT, Zall=Zc, gidx=gidx[c], emask=emask[c],
                            **shared))
    return meta, in_maps, perm


# ------------------------------------------------------------------ program

def _build(meta):
    import concourse.bass as bass
    import concourse.mybir as mybir
    from concourse import bacc
    from concourse.tile import TileContext

    DT = mybir.dt
    bf16, f32, i16 = DT.bfloat16, DT.float32, DT.int16
    AF = mybir.ActivationFunctionType
    AL = mybir.AluOpType
    AX = mybir.AxisListType

    D, chunks, S = meta["D"], meta["chunks"], meta["S"]
    tbase = meta["tbase"]
    Dmax = max(D)

    nc = bacc.Bacc("TRN2", target_bir_lowering=False, debug=False,
                   num_devices=NC)

    xT_d = nc.declare_dram_parameter("xT", [128, NL], f32, isOutput=False)
    Z_d = nc.declare_dram_parameter("Zall", [12, NL, M], f32, isOutput=False)
    W0_d = nc.declare_dram_parameter("W0", [64, L, M], bf16, isOutput=False)
    Wk_d = nc.declare_dram_parameter("Wk", [128, L, 2, 4, M], bf16, isOutput=False)
    Watt_d = nc.declare_dram_parameter("Watt", [128, L, 4, 16], bf16, isOutput=False)
    Wenc_d = nc.declare_dram_parameter("Wenc", [128, H], bf16, isOutput=False)
    We1_d = nc.declare_dram_parameter("Wevo1", [64, 128], bf16, isOutput=False)
    We2_d = nc.declare_dram_parameter("Wevo2", [128, 64], bf16, isOutput=False)
    Wdec_d = nc.declare_dram_parameter("Wdec", [64, 67], bf16, isOutput=False)
    ch_d = nc.declare_dram_parameter("chaos128", [128, H], f32, isOutput=False)
    gidx_d = nc.declare_dram_parameter("gidx", [128, 8 * S], i16, isOutput=False)
    emask_d = nc.declare_dram_parameter("emask", [128, 1, S], bf16, isOutput=False)
    out_d = nc.declare_dram_parameter("out", [128, T, 131], f32, isOutput=True)


    def reap(ap, dims):
        """rebuild AP on same tensor/offset with explicit [step, num] dims"""
        return bass.AP(ap.tensor, ap.offset, [list(ap.ap[0])] + dims)

    with TileContext(nc) as tc:
        with tc.tile_pool(name="const", bufs=1) as cpool, \
             tc.tile_pool(name="state", bufs=1) as spool, \
             tc.tile_pool(name="scratch", bufs=1) as wpool, \
             tc.tile_pool(name="zs", bufs=3) as zpool, \
             tc.tile_pool(name="wkp", bufs=1) as wkpool, \
             tc.tile_pool(name="gather", bufs=2) as gpool, \
             tc.tile_pool(name="prod", bufs=1) as ppool, \
             tc.tile_pool(name="dram", bufs=1, space="DRAM") as dpool, \
             tc.tile_pool(name="ps", bufs=2, space="PSUM") as psum, \
             tc.tile_pool(name="pss", bufs=2, space="PSUM") as psmall:

            packed_dram = dpool.tile([128, T * ROW], bf16)
            w0 = cpool.tile([64, L, M], bf16)
            nc.gpsimd.dma_start(out=w0[:], in_=W0_d[:])
            watt = cpool.tile([128, L, 4, 16], bf16)
            nc.gpsimd.dma_start(out=watt[:], in_=Watt_d[:])
            wenc = cpool.tile([128, H], bf16)
            nc.gpsimd.dma_start(out=wenc[:], in_=Wenc_d[:])
            we1 = cpool.tile([64, 128], bf16)
            nc.gpsimd.dma_start(out=we1[:], in_=We1_d[:])
            we2 = cpool.tile([128, 64], bf16)
            nc.gpsimd.dma_start(out=we2[:], in_=We2_d[:])
            wdec = cpool.tile([64, 67], bf16)
            nc.gpsimd.dma_start(out=wdec[:], in_=Wdec_d[:])
            chaos = cpool.tile([128, H], f32)
            nc.gpsimd.dma_start(out=chaos[:], in_=ch_d[:])
            gidx = cpool.tile([128, 8 * S], i16)
            nc.gpsimd.dma_start(out=gidx[:], in_=gidx_d[:])
            emask = cpool.tile([128, 1, S], bf16)
            nc.gpsimd.dma_start(out=emask[:], in_=emask_d[:])
            epst = cpool.tile([128, 1], f32)
            nc.gpsimd.memset(epst[:], float(EPS))

            h = spool.tile([128, T, H], f32)
            hsum = spool.tile([128, T, H], f32)
            nc.gpsimd.memset(hsum[:], 0.0)
            packed = spool.tile([128, T, ROW], bf16)
            hT = spool.tile([128, T, 128], bf16)
            curT0 = spool.tile([128, 4, T, 128], bf16)
            curT1 = spool.tile([128, 4, T, 128], bf16)
            sdep = spool.tile([128, T, M], f32)       # also agg / misc scratch
            cur0 = spool.tile([128, T, M], bf16)
            cur1 = spool.tile([128, T, M], bf16)
            stat = spool.tile([128, T, 8], f32)
            hnew = spool.tile([128, T, H], f32)
            hnb = spool.tile([128, T, 128], bf16)
            hnT = spool.tile([128, T, 128], bf16)
            r1b = spool.tile([128, T, 128], bf16)
            r1T = spool.tile([128, T, 128], bf16)
            gate = spool.tile([128, T, H], f32)
            den = spool.tile([128, T, 8], f32)
            exm = spool.tile([128, 8, Dmax], bf16)
            agg = spool.tile([128, T, M], f32)

            sm = wpool.tile([128, T], f32, tag="sm")
            sq = wpool.tile([128, T], f32, tag="sq")
            inv = wpool.tile([128, T], f32, tag="inv")
            nb = wpool.tile([128, T], f32, tag="nb")
            tmp2 = wpool.tile([128, T], f32, tag="tmp2")
            scr = wpool.tile([128, M], f32, tag="scr")
            xtb = wpool.tile([128, NL], bf16, tag="xtb")
            hb = wpool.tile([128, T, 128], bf16, tag="hb")
            alph = wpool.tile([128, 8, KCH], f32, tag="alph")
            at = wpool.tile([128, M], f32, tag="at")
            m1 = wpool.tile([128, 8, H], f32, tag="m1")
            hmb = wpool.tile([128, T, 128], bf16, tag="hmb")
            hmT2 = wpool.tile([128, T, 128], bf16, tag="hmT2")

            nc.gpsimd.memset(hb[:], 0.0)
            nc.gpsimd.memset(hnb[:], 0.0)
            nc.gpsimd.memset(hmb[:], 0.0)

            def ts(t):
                return slice(t * 128, (t + 1) * 128)

            def ln_finalize(width, smA, sqA, invA, nbA):
                nc.vector.tensor_scalar_mul(out=tmp2[:, :smA.shape[-1]],
                                            in0=smA, scalar1=1.0 / width)
                nc.vector.tensor_scalar_mul(out=sqA, in0=sqA, scalar1=1.0 / width)
                tm = tmp2[:, :smA.shape[-1]]
                nc.vector.scalar_tensor_tensor(out=invA, in0=tm, scalar=-1.0,
                                               op0=AL.mult, op1=AL.mult, in1=tm)
                nc.vector.tensor_add(out=invA, in0=sqA, in1=invA)
                nc.scalar.activation(out=invA, in_=invA, func=AF.Sqrt,
                                     bias=epst[:])
                nc.vector.reciprocal(out=invA, in_=invA)
                nc.vector.scalar_tensor_tensor(out=nbA, in0=tm, scalar=-1.0,
                                               op0=AL.mult, op1=AL.mult, in1=invA)

            def lhs64(tile, t):
                return tile[0:64, t, :]

            # ---------------- encoder ----------------
            nc.gpsimd.dma_start(out=xtb[:], in_=xT_d[:])
            es = sdep  # [128, T, 512]; use [:, :, :64]
            for t in range(T):
                pe = psmall.tile([128, H], f32, tag="pscp")
                nc.tensor.matmul(pe[:], xtb[:, ts(t)], wenc[:],
                                 start=True, stop=True)
                nc.vector.tensor_copy(out=es[:, t, 0:H], in_=pe[:])
                nc.vector.tensor_reduce(out=sm[:, t:t + 1], in_=es[:, t, 0:H],
                                        axis=AX.X, op=AL.add)
                nc.scalar.activation(out=scr[:, 0:H], in_=es[:, t, 0:H],
                                     func=AF.Square, accum_out=sq[:, t:t + 1])
            ln_finalize(H, sm[:], sq[:], inv[:], nb[:])
            for t in range(T):
                nc.scalar.activation(out=h[:, t, :], in_=es[:, t, 0:H],
                                     func=AF.Relu, bias=nb[:, t:t + 1],
                                     scale=inv[:, t:t + 1])
            chv = reap(chaos[:], [[0, T], list(chaos.ap[1])])
            nc.vector.tensor_tensor(out=h[:], in0=h[:], in1=chv, op=AL.add)

            # ---------------- layers ----------------
            for i in range(L):
                wk = wkpool.tile([128, 2, 4, M], bf16, tag="wk")
                nc.gpsimd.dma_start(out=wk[:], in_=Wk_d[:, i, :, :, :])

                nc.scalar.activation(out=hb[:, :, 0:H], in_=h[:], func=AF.Copy)
                nc.sync.dma_start_transpose(hT[:], reap(hb[:], [[1, T * 128]]))

                for d in range(DEPTH):
                    curD = (cur0, cur1, cur1)[d]
                    for t in range(T):
                        pc = psum.tile([128, M], f32, tag="pmm")
                        if d == 0:
                            nc.tensor.matmul(pc[:], lhs64(hT, t), w0[:, i, :],
                                             start=True, stop=True)
                        else:
                            srcT = curT0 if d == 1 else curT1
                            for ko in range(4):
                                nc.tensor.matmul(pc[:], srcT[:, ko, t, :],
                                                 wk[:, d - 1, ko, :],
                                                 start=(ko == 0), stop=(ko == 3))
                        if d == 0:
                            nc.vector.tensor_copy(out=sdep[:, t, :], in_=pc[:])
                        else:
                            zt = zpool.tile([128, M], f32, tag="z")
                            nc.gpsimd.dma_start(
                                out=zt[:], in_=Z_d[i * 2 + (d - 1), ts(t), :])
                            nc.vector.tensor_tensor(out=sdep[:, t, :],
                                                    in0=pc[:], in1=zt[:],
                                                    op=AL.add)
                        nc.vector.tensor_reduce(out=stat[:, t, 0:1],
                                                in_=sdep[:, t, :],
                                                axis=AX.X, op=AL.add)
                        nc.scalar.activation(out=scr[:], in_=sdep[:, t, :],
                                             func=AF.Square,
                                             accum_out=stat[:, t, 1:2])
                    ln_finalize(M, stat[:, :, 0], stat[:, :, 1],
                                stat[:, :, 2], stat[:, :, 3])
                    for t in range(T):
                        nc.scalar.activation(out=curD[:, t, :],
                                             in_=sdep[:, t, :], func=AF.Relu,
                                             bias=stat[:, t, 3:4],
                                             scale=stat[:, t, 2:3])
                        if d == 0:
                            nc.sync.dma_start_transpose(curT0[:, :, t, :],
                                                        curD[:, t, :])
                        elif d == 1:
                            nc.sync.dma_start_transpose(curT1[:, :, t, :],
                                                        curD[:, t, :])
                    if d == 1:
                        # cur0 := cur0 + cur1 (cur0T already consumed next depth
                        # uses curT1; safe WAR handled by Tile)
                        nc.vector.tensor_add(out=cur0[:], in0=cur0[:],
                                             in1=cur1[:])
                # hm = (cur0 + cur2)/3 ; cur1 holds cur2 now
                nc.vector.tensor_add(out=cur0[:], in0=cur0[:], in1=cur1[:])
                nc.scalar.activation(out=packed[:, :, 0:M], in_=cur0[:],
                                     func=AF.Copy, scale=1.0 / DEPTH)
                for t in range(T):
                    nc.sync.dma_start_transpose(curT0[:, :, t, :],
                                                packed[:, t, 0:M])
                for t in range(T):
                    pa = psmall.tile([128, 16], f32, tag="pscp")
                    for ko in range(4):
                        nc.tensor.matmul(pa[:], curT0[:, ko, t, :],
                                         watt[:, i, ko, :],
                                         start=(ko == 0), stop=(ko == 3))
                    nc.scalar.activation(out=packed[:, t, M:M + 16],
                                         in_=pa[:], func=AF.Copy)

                table_dram = dpool.tile([NC * NL, ROW], bf16,
                                        addr_space="Shared", tag="table",
                                        bufs=L, name="table_dram")
                nc.gpsimd.dma_start(out=packed_dram[:], in_=packed[:])
                if not _NO_COLL:
                    nc.gpsimd.collective_compute(
                        "AllGather", AL.bypass,
                        ins=[packed_dram.opt()], outs=[table_dram.opt()],
                        replica_groups=[list(range(NC))])
                else:
                    nc.gpsimd.dma_start(
                        out=table_dram[0:NL, :].rearrange("(p t) f -> p (t f)", p=128),
                        in_=packed[:])

                # ------------ edge phase ------------
                col16 = 0
                for t in range(T):
                    for (tt, k0, kc) in chunks:
                        if tt != t:
                            continue
                        a_i = reap(packed[:, t, M:M + 8], [[1, 8], [0, kc]])
                        g = gpool.tile([128, KCH, ROW], bf16, tag="g")
                        nidx = 128 * kc
                        if not _NO_GATHER:
                            nc.gpsimd.dma_gather(
                                out_ap=g[:, 0:kc, :], in_ap=table_dram[:],
                                idxs_ap=gidx[:, col16:col16 + nidx // 16],
                                num_idxs=nidx, num_idxs_reg=nidx,
                                elem_size=ROW, single_packet=False)
                        else:
                            nc.gpsimd.dma_start(
                                out=g[:, 0:kc, :],
                                in_=table_dram[t * 128:t * 128 + 128 * kc, :]
                                .rearrange("(k p) f -> p k f", p=128))
                        col16 += nidx // 16
                        gs = g[:, 0:kc, :]
                        aje = reap(gs, [[1, 8], [ROW, kc]])
                        aje = bass.AP(aje.tensor, aje.offset + (M + 8),
                                      aje.ap)
                        nc.vector.tensor_tensor(out=alph[:, :, 0:kc],
                                                in0=aje, in1=a_i, op=AL.add)
                        nc.vector.scalar_tensor_tensor(
                            out=alph[:, :, 0:kc], in0=alph[:, :, 0:kc],
                            scalar=0.2, in1=alph[:, :, 0:kc],
                            op0=AL.mult, op1=AL.max)
                        nc.scalar.activation(out=exm[:, :, k0:k0 + kc],
                                             in_=alph[:, :, 0:kc], func=AF.Exp)
                        mk = reap(emask[:, :, tbase[t] + k0:tbase[t] + k0 + kc],
                                  [[0, 8], [1, kc]])
                        nc.vector.tensor_tensor(out=exm[:, :, k0:k0 + kc],
                                                in0=exm[:, :, k0:k0 + kc],
                                                in1=mk, op=AL.mult)
                        hm_ap = reap(gs, [[ROW, kc], [H, 8], [1, H]])
                        exs = exm[:, :, k0:k0 + kc]
                        ex_ap = bass.AP(exs.tensor, exs.offset,
                                        [list(exs.ap[0]), [1, kc],
                                         [exm.ap[1][0], 8], [0, H]])
                        pr = ppool.tile([128, KCH, M], bf16, tag="pr")
                        pr_ap = reap(pr[:, 0:kc, :], [[M, kc], [H, 8], [1, H]])
                        nc.vector.tensor_tensor(out=pr_ap, in0=hm_ap,
                                                in1=ex_ap, op=AL.mult)
                        rd_ap = reap(pr[:, 0:kc, :], [[1, M], [M, kc]])
                        if k0 == 0 and kc == D[t]:
                            nc.vector.tensor_reduce(out=agg[:, t, :],
                                                    in_=rd_ap, axis=AX.X,
                                                    op=AL.add)
                        elif k0 == 0:
                            nc.vector.tensor_reduce(out=agg[:, t, :],
                                                    in_=rd_ap, axis=AX.X,
                                                    op=AL.add)
                        else:
                            nc.vector.tensor_reduce(out=at[:], in_=rd_ap,
                                                    axis=AX.X, op=AL.add)
                            nc.vector.tensor_add(out=agg[:, t, :],
                                                 in0=agg[:, t, :], in1=at[:])
                    nc.vector.tensor_reduce(out=den[:, t, :],
                                            in_=exm[:, :, 0:D[t]],
                                            axis=AX.X, op=AL.add)
                nc.vector.tensor_scalar_add(out=den[:], in0=den[:],
                                            scalar1=1e-20)
                nc.vector.reciprocal(out=den[:], in_=den[:])
                for t in range(T):
                    agg_ap = reap(agg[:, t, :], [[H, 8], [1, H]])
                    den_ap = reap(den[:, t, :], [[1, 8], [0, H]])
                    nc.vector.scalar_tensor_tensor(
                        out=m1[:], in0=agg_ap, scalar=1.0 / HEADS,
                        in1=den_ap, op0=AL.mult, op1=AL.mult)
                    rd = reap(m1[:], [[1, H], [H, 8]])
                    nc.vector.tensor_reduce(out=hnew[:, t, :], in_=rd,
                                            axis=AX.X, op=AL.add)

                # ------------ evo gate + blend + plain LN ------------
                nc.scalar.activation(out=hnb[:, :, 0:H], in_=hnew[:], func=AF.Copy)
                nc.sync.dma_start_transpose(hnT[:], reap(hnb[:], [[1, T * 128]]))
                for t in range(T):
                    pr1 = psmall.tile([128, 128], f32, tag="pscp")
                    nc.tensor.matmul(pr1[:], lhs64(hnT, t), we1[:],
                                     start=True, stop=True)
                    nc.scalar.activation(out=r1b[:, t, :], in_=pr1[:],
                                         func=AF.Relu)
                nc.sync.dma_start_transpose(r1T[:], reap(r1b[:], [[1, T * 128]]))
                for t in range(T):
                    pg = psmall.tile([128, H], f32, tag="pscp")
                    nc.tensor.matmul(pg[:], r1T[:, t, :], we2[:],
                                     start=True, stop=True)
                    nc.scalar.activation(out=gate[:, t, :], in_=pg[:],
                                         func=AF.Tanh)
                nc.scalar.activation(out=gate[:], in_=gate[:], func=AF.Sigmoid)
                dlt = sdep  # reuse [:, :, 0:H] as scratch
                nc.vector.tensor_sub(out=dlt[:, :, 0:H], in0=hnew[:], in1=h[:])
                nc.vector.tensor_tensor(out=dlt[:, :, 0:H], in0=dlt[:, :, 0:H],
                                        in1=gate[:], op=AL.mult)
                nc.vector.tensor_add(out=h[:], in0=h[:], in1=dlt[:, :, 0:H])
                nc.vector.tensor_add(out=hsum[:], in0=hsum[:], in1=h[:])
                nc.vector.tensor_reduce(out=sm[:], in_=h[:], axis=AX.X,
                                        op=AL.add)
                nc.vector.tensor_mul(out=dlt[:, :, H:2 * H], in0=h[:],
                                     in1=h[:])
                nc.vector.tensor_reduce(out=sq[:], in_=dlt[:, :, H:2 * H],
                                        axis=AX.X, op=AL.add)
                ln_finalize(H, sm[:], sq[:], inv[:], nb[:])
                nc.vector.tensor_tensor(out=h[:], in0=h[:],
                                        in1=reap(inv[:], [[1, T], [0, H]]),
                                        op=AL.mult)
                nc.vector.tensor_tensor(out=h[:], in0=h[:],
                                        in1=reap(nb[:], [[1, T], [0, H]]),
                                        op=AL.add)

            # ---------------- decoders ----------------
            hmf = sdep  # [:, :, 0:H]
            nc.scalar.activation(out=hmf[:, :, 0:H], in_=hsum[:], func=AF.Copy,
                                 scale=1.0 / L)
            nc.scalar.activation(out=hmb[:, :, 0:H], in_=hmf[:, :, 0:H], func=AF.Copy)
            nc.sync.dma_start_transpose(hmT2[:], reap(hmb[:], [[1, T * 128]]))
            ob = spool.tile([128, T, 131], f32)
            for t in range(T):
                pdc = psmall.tile([128, 67], f32, tag="pscp")
                nc.tensor.matmul(pdc[:], lhs64(hmT2, t), wdec[:],
                                 start=True, stop=True)
                nc.vector.tensor_copy(out=ob[:, t, 0:67], in_=pdc[:])
            nc.vector.tensor_copy(out=ob[:, :, 67:131], in_=hmf[:, :, 0:H])
            nc.gpsimd.dma_start(out=out_d[:], in_=ob[:])

    nc.compile()
    return nc


# ------------------------------------------------------------------ entry

def kernel(**inputs):
    from concourse.bass_utils import run_bass_kernel_spmd

    meta, in_maps, perm = _prep(inputs)
    key = tuple(meta["D"])
    if key not in _CACHE:
        _CACHE[key] = _build(meta)
    nc = _CACHE[key]
    res = run_bass_kernel_spmd(nc, in_maps, list(range(NC))).results

    out = np.zeros((N, 131), np.float32)
    for c in range(NC):
        o = np.asarray(res[c]["out"]).reshape(128, T, 131)
        o = o.transpose(1, 0, 2).reshape(NL, 131)
        out[perm[c]] = o[:NREAL]
    return (out[:, 0:1], out[:, 1:2], out[:, 2:3],
            out[:, 3:67], out[:, 67:131])
